# revision 1
# baseline (speedup 1.0000x reference)
"""Trainium2 Bass kernel for nn_CCSOFT (SO(3) cross-correlation via SOFT).

Math (validated vs reference):
  wig[l,m,k,n] factors as d[l,m,k]*d[l,k,n]  (rank-1 in (m,n) per (l,k)).
  Recover u[l,k,m]=d[l,m,k], v[l,k,n]=d[l,k,n] on host from wig, then fuse the
  lmkn contraction with the 3D inverse DFT (127 is prime -> DFT = matmul):
    E[x,m] = exp(+2j*pi*m*x/127)/127          (same matrix for all 3 axes)
    P[b,l,m,k] = F[b,l,m]*u[l,k,m]            F = f_re + i f_im
    A[b,l,x,k] = sum_m E[x,m] P[b,l,m,k]      (stage 1)
    Q[b,l,n,k] = G[b,l,n]*v[l,k,n]            G = conj(g)
    C[b,l,k,z] = sum_n E[z,n] Q[b,l,n,k]      (stage 2)
    S[b,k,x,z] = sum_l A[b,l,x,k] C[b,l,k,z]  (stage 3, contract (re/im,l)=128)
    out[b,x,y,z] = sum_k E[y,k] S[b,k,x,z]    (stage 4)

Data parallel over batch b: 32 batches -> 4 per core on 8 NeuronCores.
Layout rotations (to get l resp. k onto the partition axis) bounce through
DRAM scratch in bf16. All matmuls bf16 x bf16 -> fp32 PSUM.
"""

import sys

if "/opt/trn_rl_repo" not in sys.path:
    sys.path.insert(0, "/opt/trn_rl_repo")

import ml_dtypes
import numpy as np

import concourse.tile as tile
from concourse import bacc, mybir
from concourse.bass_utils import run_bass_kernel_spmd

B, L, M = 32, 64, 127
NCORES = 8
BC = B // NCORES          # batches per core
NJ = 16                   # stage1/2 chunks: 16 chunks x 4 l's x 127 k-cols
LJ = L // NJ              # 4
CH = LJ * M               # 508 columns per chunk
BF16 = mybir.dt.float16  # fp16: 10-bit mantissa
F32 = mybir.dt.float32
NPBF = np.float16

_PROG = None  # cached (nc, meta)


def _factor_wig(wig):
    """wig (L,M,M,M) float32 -> u[l,k,m], v[l,k,n] with u*v^T == wig[l,:,k,:]."""
    R = np.ascontiguousarray(wig.transpose(0, 2, 1, 3))          # (l,k,m,n)
    Rf = R.reshape(L, M, M * M)
    idx = np.abs(Rf).argmax(-1)
    mstar, nstar = idx // M, idx % M
    s = np.take_along_axis(Rf, idx[..., None], -1)[..., 0]       # R[l,k,m*,n*]
    u = np.take_along_axis(R, nstar[..., None, None], 3)[..., 0]  # (l,k,m)
    v = np.take_along_axis(R, mstar[..., None, None], 2)[..., 0, :]  # (l,k,n)
    safe = np.abs(s) > 0
    v = np.where(safe[..., None], v / np.where(safe, s, 1)[..., None], 0.0)
    u = np.where(safe[..., None], u, 0.0)
    return u.astype(np.float32), v.astype(np.float32)


def _build_program():
    nc = bacc.Bacc("TRN2", target_bir_lowering=False, debug=False,
                   num_devices=NCORES)

    # ---- external inputs (per core) ----
    # d-factor stacks: dstkM[m, l, k] = u[l,k,m]; dstkT[n, l, k] = v[l,k,n]
    dstkM = nc.dram_tensor("dstkM", [M, L, M], BF16, kind="ExternalInput").ap()
    dstkT = nc.dram_tensor("dstkT", [M, L, M], BF16, kind="ExternalInput").ap()
    # DFT matrices (symmetric): ex_re[m,x]=cos(2pi m x/127)/127, ex_im=sin/127
    exre_d = nc.dram_tensor("exre", [M, M], BF16, kind="ExternalInput").ap()
    exim_d = nc.dram_tensor("exim", [M, M], BF16, kind="ExternalInput").ap()
    eximn_d = nc.dram_tensor("eximn", [M, M], BF16, kind="ExternalInput").ap()
    # f/g tiles: [m, b, l] with b = per-core batch index; gimn = -g_im (conj)
    fre_d = nc.dram_tensor("fre", [M, BC, L], BF16, kind="ExternalInput").ap()
    fim_d = nc.dram_tensor("fim", [M, BC, L], BF16, kind="ExternalInput").ap()
    gre_d = nc.dram_tensor("gre", [M, BC, L], BF16, kind="ExternalInput").ap()
    gimn_d = nc.dram_tensor("gimn", [M, BC, L], BF16, kind="ExternalInput").ap()

    # ---- external outputs ----
    outre = nc.dram_tensor("outre", [BC, M, M, M], BF16, kind="ExternalOutput").ap()  # [b,y,x,z]
    outim = nc.dram_tensor("outim", [BC, M, M, M], BF16, kind="ExternalOutput").ap()  # [b,y,x,z]

    # ---- DRAM scratch (bounce buffers, bf16) ----
    # A: [c, j, x, l4, k]; C: [c(re,im,imneg), j, z, l4, k]; S: [k, x, c, z]
    Adram_t = nc.dram_tensor("Adram", [BC, 2, L, M, M], BF16).ap()
    Cdram_t = nc.dram_tensor("Cdram", [BC, 3, L, M, M], BF16).ap()
    Sdram_t = nc.dram_tensor("Sdram", [BC, M, 2, M, M], BF16).ap()

    from contextlib import ExitStack
    with tile.TileContext(nc) as tc, ExitStack() as ctx:
        cpool = ctx.enter_context(tc.tile_pool(name="consts", bufs=1))
        pool1 = ctx.enter_context(tc.tile_pool(name="pq_astk", bufs=2))
        pool2 = ctx.enter_context(tc.tile_pool(name="cstk_sk", bufs=2))
        scr1 = ctx.enter_context(tc.tile_pool(name="scr1", bufs=4))
        scr3 = ctx.enter_context(tc.tile_pool(name="scr3", bufs=3))
        psp = ctx.enter_context(tc.tile_pool(name="psp", bufs=4, space="PSUM"))
        ps1 = ps3 = ps4 = psp
        scr4 = ctx.enter_context(tc.tile_pool(name="scr4", bufs=4))

        # constants into SBUF
        dM = cpool.tile([M, L * M], BF16, tag="dM")
        nc.sync.dma_start(dM[:], dstkM.rearrange("m l k -> m (l k)"))
        dT = cpool.tile([M, L * M], BF16, tag="dT")
        nc.sync.dma_start(dT[:], dstkT.rearrange("n l k -> n (l k)"))
        exre = cpool.tile([M, M], BF16, tag="exre")
        nc.sync.dma_start(exre[:], exre_d)
        exim = cpool.tile([M, M], BF16, tag="exim")
        nc.sync.dma_start(exim[:], exim_d)
        eximn = cpool.tile([M, M], BF16, tag="eximn")
        nc.sync.dma_start(eximn[:], eximn_d)
        fgt = {}
        for nm, dr in (("fre", fre_d), ("fim", fim_d), ("gre", gre_d),
                       ("gimn", gimn_d)):
            t = cpool.tile([M, BC * L], BF16, tag=nm)
            nc.sync.dma_start(t[:], dr.rearrange("m b l -> m (b l)"))
            fgt[nm] = t

        for b in range(BC):
            Adram = Adram_t[b]
            Cdram = Cdram_t[b]
            Sdram = Sdram_t[b]
            # ============ stage 2: Q build + C = E @ Q ============
            Q = pool1.tile([M, 2 * L * M], BF16, tag="pq")
            Q3 = Q[:].rearrange("n (c l k) -> n c l k", c=2, l=L)
            dT3 = dT[:].rearrange("n (l k) -> n l k", l=L)
            for ci, nm in enumerate(("gre", "gimn")):
                gb = fgt[nm][:, b * L:(b + 1) * L]
                nc.vector.tensor_tensor(
                    out=Q3[:, ci], in0=dT3, in1=gb.broadcast_to((M, L, M)),
                    op=mybir.AluOpType.mult)
            csc = {}
            for j in range(NJ):
                rre = Q[:, j * CH:(j + 1) * CH]
                rim = Q[:, L * M + j * CH:L * M + (j + 1) * CH]
                pc_re = ps1.tile([M, 1024], F32, tag="ps")
                nc.tensor.matmul(pc_re[:, 0:CH], exre[:], rre, start=True, stop=False)
                nc.tensor.matmul(pc_re[:, 0:CH], eximn[:], rim, start=False, stop=True)
                pc_im = ps1.tile([M, 1024], F32, tag="ps")
                nc.tensor.matmul(pc_im[:, 0:CH], exim[:], rre, start=True, stop=False)
                nc.tensor.matmul(pc_im[:, 0:CH], exre[:], rim, start=False, stop=True)
                for ci, ps, scl in ((0, pc_re, 1.0), (1, pc_im, 1.0),
                                    (2, pc_im, -1.0)):
                    if j % 2 == 0:
                        csc[ci] = scr1.tile([M, 2 * CH], BF16, tag="scr1", name=f"csc{ci}")
                    half = csc[ci][:, (j % 2) * CH:(j % 2 + 1) * CH]
                    if ci == 0:
                        nc.vector.tensor_copy(half, ps[:, 0:CH])
                    else:
                        nc.scalar.mul(half, ps[:, 0:CH], scl)
                    if j % 2 == 1:
                        nc.sync.dma_start(
                            Cdram[ci, (j - 1) * LJ:(j + 1) * LJ].rearrange(
                                "l z k -> z l k"),
                            csc[ci][:].rearrange("z (l k) -> z l k", l=2 * LJ))

            # prefetch C stacks during stage 1 (depend only on Cdram writes)
            CstkRe = pool2.tile([2 * L, M * M], BF16, tag="cstk")  # [C_re; -C_im]
            CstkIm = pool2.tile([2 * L, M * M], BF16, tag="cstk")  # [C_im;  C_re]
            nc.gpsimd.dma_start(
                CstkRe[0:L].rearrange("l (z k) -> l z k", z=M), Cdram[0])
            nc.gpsimd.dma_start(
                CstkRe[L:2 * L].rearrange("l (z k) -> l z k", z=M), Cdram[2])
            nc.gpsimd.dma_start(
                CstkIm[0:L].rearrange("l (z k) -> l z k", z=M), Cdram[1])
            nc.gpsimd.dma_start(
                CstkIm[L:2 * L].rearrange("l (z k) -> l z k", z=M), Cdram[0])

            # ============ stage 1: P build + A = E @ P ============
            P = pool1.tile([M, 2 * L * M], BF16, tag="pq")
            P3 = P[:].rearrange("m (c l k) -> m c l k", c=2, l=L)
            d3 = dM[:].rearrange("m (l k) -> m l k", l=L)
            for ci, nm in enumerate(("fre", "fim")):
                fb = fgt[nm][:, b * L:(b + 1) * L]          # [m, l]
                nc.vector.tensor_tensor(
                    out=P3[:, ci], in0=d3, in1=fb.broadcast_to((M, L, M)),
                    op=mybir.AluOpType.mult)
            asc = {}
            for j in range(NJ):
                rre = P[:, j * CH:(j + 1) * CH]
                rim = P[:, L * M + j * CH:L * M + (j + 1) * CH]
                pa_re = ps1.tile([M, 1024], F32, tag="ps")
                nc.tensor.matmul(pa_re[:, 0:CH], exre[:], rre, start=True, stop=False)
                nc.tensor.matmul(pa_re[:, 0:CH], eximn[:], rim, start=False, stop=True)
                pa_im = ps1.tile([M, 1024], F32, tag="ps")
                nc.tensor.matmul(pa_im[:, 0:CH], exim[:], rre, start=True, stop=False)
                nc.tensor.matmul(pa_im[:, 0:CH], exre[:], rim, start=False, stop=True)
                for ci, ps in ((0, pa_re), (1, pa_im)):
                    if j % 2 == 0:
                        asc[ci] = scr1.tile([M, 2 * CH], BF16, tag="scr1", name=f"asc{ci}")
                    half = asc[ci][:, (j % 2) * CH:(j % 2 + 1) * CH]
                    if ci == 0:
                        nc.vector.tensor_copy(half, ps[:, 0:CH])
                    else:
                        nc.scalar.mul(half, ps[:, 0:CH], 1.0)
                    if j % 2 == 1:
                        nc.sync.dma_start(
                            Adram[ci, (j - 1) * LJ:(j + 1) * LJ].rearrange(
                                "l x k -> x l k"),
                            asc[ci][:].rearrange("x (l k) -> x l k", l=2 * LJ))

            # Astk load right after stage-1 writes
            Astk = pool1.tile([2 * L, M * M], BF16, tag="pq")   # [(c,l),(x,k)]
            nc.gpsimd.dma_start(
                Astk[0:L].rearrange("l (x k) -> l x k", x=M), Adram[0])
            nc.gpsimd.dma_start(
                Astk[L:2 * L].rearrange("l (x k) -> l x k", x=M), Adram[1])

            # ============ stage 3: S[b,k] = sum_(c,l) A~ C~ ============
            A3 = Astk[:].rearrange("p (x k) -> p x k", x=M)
            CR3 = CstkRe[:].rearrange("p (z k) -> p z k", z=M)
            CI3 = CstkIm[:].rearrange("p (z k) -> p z k", z=M)
            s4 = Sdram.rearrange("k c x z -> x k c z")
            NG = (M + 1) // 2                                    # 64 k-groups
            for kg in range(NG):
                kn = min(2, M - kg * 2)
                psS = ps3.tile([M, 1024], F32, tag="ps")        # 2 banks
                for t in range(kn):
                    k = kg * 2 + t
                    nc.tensor.matmul(psS[:, t * 512:t * 512 + M],
                                     A3[:, :, k], CR3[:, :, k],
                                     start=True, stop=True)
                    nc.tensor.matmul(psS[:, t * 512 + 256:t * 512 + 256 + M],
                                     A3[:, :, k], CI3[:, :, k],
                                     start=True, stop=True)
                if kg % 2 == 0:
                    ssc = scr3.tile([M, 8 * M], BF16, tag="scr3")
                    ssc_k0 = kg * 2
                pview = psS[:].rearrange("x (t c u) -> x t c u", t=2, c=2)
                sv = ssc[:].rearrange("x (t c z) -> x t c z", t=4, c=2)
                toff = (kg % 2) * 2
                if kg % 2 == 0:
                    nc.vector.tensor_copy(sv[:, toff:toff + kn, :, 0:M],
                                          pview[:, 0:kn, :, 0:M])
                else:
                    nc.scalar.mul(sv[:, toff:toff + kn, :, 0:M],
                                  pview[:, 0:kn, :, 0:M], 1.0)
                if kg % 2 == 1 or kg == NG - 1:
                    ktot = kg * 2 + kn - ssc_k0
                    nc.sync.dma_start(
                        s4[:, ssc_k0:ssc_k0 + ktot],
                        sv[:, 0:ktot, :, 0:M])

            # ============ stage 4: out[b] = E @ S ============
            Sk0 = pool2.tile([M, M * M], BF16, tag="cstk")   # S_re [k,(x,z)]
            Sk1 = pool2.tile([M, M * M], BF16, tag="cstk")   # S_im
            nc.gpsimd.dma_start(
                Sk0[:].rearrange("k (x z) -> k x z", x=M), Sdram[:, 0])
            nc.gpsimd.dma_start(
                Sk1[:].rearrange("k (x z) -> k x z", x=M), Sdram[:, 1])
            ore = outre[b]   # [y, x, z] layout
            oim = outim[b]
            for xg in range(32):                              # groups of 4 x's
                xn = min(4, M - xg * 4)
                cw = xn * M
                c0 = Sk0[:, xg * 4 * M:xg * 4 * M + cw]
                c1 = Sk1[:, xg * 4 * M:xg * 4 * M + cw]
                if xg % 2 == 0:
                    sore = scr4.tile([M, 2 * CH], BF16, tag="scr4")
                    soim = scr4.tile([M, 2 * CH], BF16, tag="scr4")
                    so_x0 = xg * 4
                off = (xg % 2) * CH
                po = ps4.tile([M, 1024], F32, tag="ps")
                nc.tensor.matmul(po[:, 0:cw], exre[:], c0, start=True, stop=False)
                nc.tensor.matmul(po[:, 0:cw], eximn[:], c1, start=False, stop=True)
                nc.vector.tensor_copy(sore[:, off:off + cw], po[:, 0:cw])
                po = ps4.tile([M, 1024], F32, tag="ps")
                nc.tensor.matmul(po[:, 0:cw], exim[:], c0, start=True, stop=False)
                nc.tensor.matmul(po[:, 0:cw], exre[:], c1, start=False, stop=True)
                nc.scalar.mul(soim[:, off:off + cw], po[:, 0:cw], 1.0)
                if xg % 2 == 1:
                    xtot = xg * 4 + xn - so_x0
                    nc.sync.dma_start(
                        ore[:, so_x0:so_x0 + xtot],
                        sore[:, 0:off + cw].rearrange(
                            "y (x z) -> y x z", x=xtot))
                    nc.sync.dma_start(
                        oim[:, so_x0:so_x0 + xtot],
                        soim[:, 0:off + cw].rearrange(
                            "y (x z) -> y x z", x=xtot))

    nc.compile()
    return nc


def _get_program():
    global _PROG
    if _PROG is None:
        _PROG = _build_program()
    return _PROG


def _make_inmaps(f_re, f_im, g_re, g_im, wig):
    u, v = _factor_wig(np.asarray(wig, dtype=np.float32))
    # dstkM[m,l,k] = u[l,k,m]; dstkT[n,l,k] = v[l,k,n]
    dstkM = np.ascontiguousarray(u.transpose(2, 0, 1)).astype(NPBF)
    dstkT = np.ascontiguousarray(v.transpose(2, 0, 1)).astype(NPBF)
    ang = 2.0 * np.pi * np.outer(np.arange(M), np.arange(M)) / M
    exre = (np.cos(ang) / M).astype(NPBF)
    exim = (np.sin(ang) / M).astype(NPBF)
    eximn = (-np.sin(ang) / M).astype(NPBF)

    def fgt(x, sl, neg=False):
        t = np.asarray(x, dtype=np.float32)[sl]            # (BC, L, M)
        t = t.transpose(2, 0, 1)                           # (M, BC, L)
        if neg:
            t = -t
        return np.ascontiguousarray(t).astype(NPBF)

    in_maps = []
    for c in range(NCORES):
        sl = slice(c * BC, (c + 1) * BC)
        in_maps.append({
            "dstkM": dstkM, "dstkT": dstkT,
            "exre": exre, "exim": exim, "eximn": eximn,
            "fre": fgt(f_re, sl), "fim": fgt(f_im, sl),
            "gre": fgt(g_re, sl), "gimn": fgt(g_im, sl, neg=True),
        })
    return in_maps


def kernel(f_re, f_im, g_re, g_im, wig):
    nc = _get_program()
    in_maps = _make_inmaps(f_re, f_im, g_re, g_im, wig)
    res = run_bass_kernel_spmd(nc, in_maps, list(range(NCORES)))
    out = np.empty((B, M, M, M), dtype=np.complex64)
    for c in range(NCORES):
        r = res.results[c]
        # device wrote [b, y, x, z]; reference order is [b, x, y, z]
        out[c * BC:(c + 1) * BC] = (
            r["outre"].astype(np.float32).transpose(0, 2, 1, 3)
            + 1j * r["outim"].astype(np.float32).transpose(0, 2, 1, 3))
    return out



# revision 2
# speedup vs baseline: 5009.0819x; 5009.0819x over previous
"""Trainium2 Bass kernel for nn_CCSOFT (SO(3) cross-correlation via SOFT).

Math (validated vs reference):
  wig[l,m,k,n] factors as d[l,m,k]*d[l,k,n]  (rank-1 in (m,n) per (l,k)).
  Recover u[l,k,m]=d[l,m,k], v[l,k,n]=d[l,k,n] on host from wig, then fuse the
  lmkn contraction with the 3D inverse DFT (127 is prime -> DFT = matmul):
    E[x,m] = exp(+2j*pi*m*x/127)/127          (same matrix for all 3 axes)
    P[b,l,m,k] = F[b,l,m]*u[l,k,m]            F = f_re + i f_im
    A[b,l,x,k] = sum_m E[x,m] P[b,l,m,k]      (stage 1)
    Q[b,l,n,k] = G[b,l,n]*v[l,k,n]            G = conj(g)
    C[b,l,k,z] = sum_n E[z,n] Q[b,l,n,k]      (stage 2)
    S[b,k,x,z] = sum_l A[b,l,x,k] C[b,l,k,z]  (stage 3, contract (re/im,l)=128)
    out[b,x,y,z] = sum_k E[y,k] S[b,k,x,z]    (stage 4)

Data parallel over batch b: 32 batches -> 4 per core on 8 NeuronCores.
Layout rotations (to get l resp. k onto the partition axis) bounce through
DRAM scratch in bf16. All matmuls bf16 x bf16 -> fp32 PSUM.
"""

import sys

if "/opt/trn_rl_repo" not in sys.path:
    sys.path.insert(0, "/opt/trn_rl_repo")

import ml_dtypes
import numpy as np

import concourse.tile as tile
from concourse import bacc, mybir
from concourse.bass_utils import run_bass_kernel_spmd

B, L, M = 32, 64, 127
NCORES = 8
BC = B // NCORES          # batches per core
NJ = 16                   # stage1/2 chunks: 16 chunks x 4 l's x 127 k-cols
LJ = L // NJ              # 4
CH = LJ * M               # 508 columns per chunk
BF16 = mybir.dt.float16  # fp16: 10-bit mantissa
F32 = mybir.dt.float32
NPBF = np.float16

_PROG = None  # cached (nc, meta)


def _factor_wig(wig):
    """wig (L,M,M,M) float32 -> u[l,k,m], v[l,k,n] with u*v^T == wig[l,:,k,:]."""
    R = np.ascontiguousarray(wig.transpose(0, 2, 1, 3))          # (l,k,m,n)
    Rf = R.reshape(L, M, M * M)
    idx = np.abs(Rf).argmax(-1)
    mstar, nstar = idx // M, idx % M
    s = np.take_along_axis(Rf, idx[..., None], -1)[..., 0]       # R[l,k,m*,n*]
    u = np.take_along_axis(R, nstar[..., None, None], 3)[..., 0]  # (l,k,m)
    v = np.take_along_axis(R, mstar[..., None, None], 2)[..., 0, :]  # (l,k,n)
    safe = np.abs(s) > 0
    v = np.where(safe[..., None], v / np.where(safe, s, 1)[..., None], 0.0)
    u = np.where(safe[..., None], u, 0.0)
    return u.astype(np.float32), v.astype(np.float32)


def _build_program():
    nc = bacc.Bacc("TRN2", target_bir_lowering=False, debug=False,
                   num_devices=NCORES)

    # ---- external inputs (per core) ----
    # d-factor stacks: dstkM[m, l, k] = u[l,k,m]; dstkT[n, l, k] = v[l,k,n]
    dstkM = nc.dram_tensor("dstkM", [M, L, M], BF16, kind="ExternalInput").ap()
    dstkT = nc.dram_tensor("dstkT", [M, L, M], BF16, kind="ExternalInput").ap()
    # DFT matrices (symmetric): ex_re[m,x]=cos(2pi m x/127)/127, ex_im=sin/127
    exre_d = nc.dram_tensor("exre", [M, M], BF16, kind="ExternalInput").ap()
    exim_d = nc.dram_tensor("exim", [M, M], BF16, kind="ExternalInput").ap()
    eximn_d = nc.dram_tensor("eximn", [M, M], BF16, kind="ExternalInput").ap()
    # f/g tiles: [m, b, l] with b = per-core batch index; gimn = -g_im (conj)
    fre_d = nc.dram_tensor("fre", [M, BC, L], BF16, kind="ExternalInput").ap()
    fim_d = nc.dram_tensor("fim", [M, BC, L], BF16, kind="ExternalInput").ap()
    gre_d = nc.dram_tensor("gre", [M, BC, L], BF16, kind="ExternalInput").ap()
    gimn_d = nc.dram_tensor("gimn", [M, BC, L], BF16, kind="ExternalInput").ap()

    # ---- external outputs ----
    outre = nc.dram_tensor("outre", [BC, M, M, M], BF16, kind="ExternalOutput").ap()  # [b,y,x,z]
    outim = nc.dram_tensor("outim", [BC, M, M, M], BF16, kind="ExternalOutput").ap()  # [b,y,x,z]

    # ---- DRAM scratch (bounce buffers, bf16) ----
    # A: [c, j, x, l4, k]; C: [c(re,im,imneg), j, z, l4, k]; S: [k, x, c, z]
    Adram_t = nc.dram_tensor("Adram", [BC, 2, L, M, M], BF16).ap()
    Cdram_t = nc.dram_tensor("Cdram", [BC, 3, L, M, M], BF16).ap()
    Sdram_t = nc.dram_tensor("Sdram", [BC, M, 2, M, M], BF16).ap()

    from contextlib import ExitStack
    with tile.TileContext(nc) as tc, ExitStack() as ctx:
        cpool = ctx.enter_context(tc.tile_pool(name="consts", bufs=1))
        pool1 = ctx.enter_context(tc.tile_pool(name="pq_astk", bufs=2))
        pool2 = ctx.enter_context(tc.tile_pool(name="cstk_sk", bufs=2))
        scr1 = ctx.enter_context(tc.tile_pool(name="scr1", bufs=4))
        scr3 = ctx.enter_context(tc.tile_pool(name="scr3", bufs=3))
        psp = ctx.enter_context(tc.tile_pool(name="psp", bufs=4, space="PSUM"))
        ps1 = ps3 = ps4 = psp
        scr4 = ctx.enter_context(tc.tile_pool(name="scr4", bufs=4))

        # constants into SBUF
        dM = cpool.tile([M, L * M], BF16, tag="dM")
        nc.sync.dma_start(dM[:], dstkM.rearrange("m l k -> m (l k)"))
        dT = cpool.tile([M, L * M], BF16, tag="dT")
        nc.sync.dma_start(dT[:], dstkT.rearrange("n l k -> n (l k)"))
        exre = cpool.tile([M, M], BF16, tag="exre")
        nc.sync.dma_start(exre[:], exre_d)
        exim = cpool.tile([M, M], BF16, tag="exim")
        nc.sync.dma_start(exim[:], exim_d)
        eximn = cpool.tile([M, M], BF16, tag="eximn")
        nc.sync.dma_start(eximn[:], eximn_d)
        fgt = {}
        for nm, dr in (("fre", fre_d), ("fim", fim_d), ("gre", gre_d),
                       ("gimn", gimn_d)):
            t = cpool.tile([M, BC * L], BF16, tag=nm)
            nc.sync.dma_start(t[:], dr.rearrange("m b l -> m (b l)"))
            fgt[nm] = t

        for b in range(BC):
            Adram = Adram_t[b]
            Cdram = Cdram_t[b]
            Sdram = Sdram_t[b]
            # ============ stage 2: Q build + C = E @ Q ============
            Q = pool1.tile([M, 2 * L * M], BF16, tag="pq")
            Q3 = Q[:].rearrange("n (c l k) -> n c l k", c=2, l=L)
            dT3 = dT[:].rearrange("n (l k) -> n l k", l=L)
            for ci, nm in enumerate(("gre", "gimn")):
                gb = fgt[nm][:, b * L:(b + 1) * L]
                nc.vector.tensor_tensor(
                    out=Q3[:, ci], in0=dT3, in1=gb.broadcast_to((M, L, M)),
                    op=mybir.AluOpType.mult)
            csc = {}
            for j in range(NJ):
                rre = Q[:, j * CH:(j + 1) * CH]
                rim = Q[:, L * M + j * CH:L * M + (j + 1) * CH]
                pc_re = ps1.tile([M, 1024], F32, tag="ps")
                nc.tensor.matmul(pc_re[:, 0:CH], exre[:], rre, start=True, stop=False)
                nc.tensor.matmul(pc_re[:, 0:CH], eximn[:], rim, start=False, stop=True)
                pc_im = ps1.tile([M, 1024], F32, tag="ps")
                nc.tensor.matmul(pc_im[:, 0:CH], exim[:], rre, start=True, stop=False)
                nc.tensor.matmul(pc_im[:, 0:CH], exre[:], rim, start=False, stop=True)
                for ci, ps, scl in ((0, pc_re, 1.0), (1, pc_im, 1.0),
                                    (2, pc_im, -1.0)):
                    if j % 2 == 0:
                        csc[ci] = scr1.tile([M, 2 * CH], BF16, tag="scr1", name=f"csc{ci}")
                    half = csc[ci][:, (j % 2) * CH:(j % 2 + 1) * CH]
                    if ci == 0:
                        nc.vector.tensor_copy(half, ps[:, 0:CH])
                    else:
                        nc.scalar.mul(half, ps[:, 0:CH], scl)
                    if j % 2 == 1:
                        nc.sync.dma_start(
                            Cdram[ci, (j - 1) * LJ:(j + 1) * LJ].rearrange(
                                "l z k -> z l k"),
                            csc[ci][:].rearrange("z (l k) -> z l k", l=2 * LJ))

            # prefetch C stacks during stage 1 (depend only on Cdram writes)
            CstkRe = pool2.tile([2 * L, M * M], BF16, tag="cstk")  # [C_re; -C_im]
            CstkIm = pool2.tile([2 * L, M * M], BF16, tag="cstk")  # [C_im;  C_re]
            nc.gpsimd.dma_start(
                CstkRe[0:L].rearrange("l (z k) -> l z k", z=M), Cdram[0])
            nc.gpsimd.dma_start(
                CstkRe[L:2 * L].rearrange("l (z k) -> l z k", z=M), Cdram[2])
            nc.gpsimd.dma_start(
                CstkIm[0:L].rearrange("l (z k) -> l z k", z=M), Cdram[1])
            nc.gpsimd.dma_start(
                CstkIm[L:2 * L].rearrange("l (z k) -> l z k", z=M), Cdram[0])

            # ============ stage 1: P build + A = E @ P ============
            P = pool1.tile([M, 2 * L * M], BF16, tag="pq")
            P3 = P[:].rearrange("m (c l k) -> m c l k", c=2, l=L)
            d3 = dM[:].rearrange("m (l k) -> m l k", l=L)
            for ci, nm in enumerate(("fre", "fim")):
                fb = fgt[nm][:, b * L:(b + 1) * L]          # [m, l]
                nc.vector.tensor_tensor(
                    out=P3[:, ci], in0=d3, in1=fb.broadcast_to((M, L, M)),
                    op=mybir.AluOpType.mult)
            asc = {}
            for j in range(NJ):
                rre = P[:, j * CH:(j + 1) * CH]
                rim = P[:, L * M + j * CH:L * M + (j + 1) * CH]
                pa_re = ps1.tile([M, 1024], F32, tag="ps")
                nc.tensor.matmul(pa_re[:, 0:CH], exre[:], rre, start=True, stop=False)
                nc.tensor.matmul(pa_re[:, 0:CH], eximn[:], rim, start=False, stop=True)
                pa_im = ps1.tile([M, 1024], F32, tag="ps")
                nc.tensor.matmul(pa_im[:, 0:CH], exim[:], rre, start=True, stop=False)
                nc.tensor.matmul(pa_im[:, 0:CH], exre[:], rim, start=False, stop=True)
                for ci, ps in ((0, pa_re), (1, pa_im)):
                    if j % 2 == 0:
                        asc[ci] = scr1.tile([M, 2 * CH], BF16, tag="scr1", name=f"asc{ci}")
                    half = asc[ci][:, (j % 2) * CH:(j % 2 + 1) * CH]
                    if ci == 0:
                        nc.vector.tensor_copy(half, ps[:, 0:CH])
                    else:
                        nc.scalar.mul(half, ps[:, 0:CH], 1.0)
                    if j % 2 == 1:
                        nc.sync.dma_start(
                            Adram[ci, (j - 1) * LJ:(j + 1) * LJ].rearrange(
                                "l x k -> x l k"),
                            asc[ci][:].rearrange("x (l k) -> x l k", l=2 * LJ))

            # Astk load right after stage-1 writes
            Astk = pool1.tile([2 * L, M * M], BF16, tag="pq")   # [(c,l),(x,k)]
            nc.gpsimd.dma_start(
                Astk[0:L].rearrange("l (x k) -> l x k", x=M), Adram[0])
            nc.gpsimd.dma_start(
                Astk[L:2 * L].rearrange("l (x k) -> l x k", x=M), Adram[1])

            # ============ stage 3: S[b,k] = sum_(c,l) A~ C~ ============
            A3 = Astk[:].rearrange("p (x k) -> p x k", x=M)
            CR3 = CstkRe[:].rearrange("p (z k) -> p z k", z=M)
            CI3 = CstkIm[:].rearrange("p (z k) -> p z k", z=M)
            s4 = Sdram.rearrange("k c x z -> x k c z")
            NG = (M + 1) // 2                                    # 64 k-groups
            for kg in range(NG):
                kn = min(2, M - kg * 2)
                psS = ps3.tile([M, 1024], F32, tag="ps")        # 2 banks
                for t in range(kn):
                    k = kg * 2 + t
                    nc.tensor.matmul(psS[:, t * 512:t * 512 + M],
                                     A3[:, :, k], CR3[:, :, k],
                                     start=True, stop=True)
                    nc.tensor.matmul(psS[:, t * 512 + 256:t * 512 + 256 + M],
                                     A3[:, :, k], CI3[:, :, k],
                                     start=True, stop=True)
                if kg % 2 == 0:
                    ssc = scr3.tile([M, 8 * M], BF16, tag="scr3")
                    ssc_k0 = kg * 2
                pview = psS[:].rearrange("x (t c u) -> x t c u", t=2, c=2)
                sv = ssc[:].rearrange("x (t c z) -> x t c z", t=4, c=2)
                toff = (kg % 2) * 2
                if kg % 2 == 0:
                    nc.vector.tensor_copy(sv[:, toff:toff + kn, :, 0:M],
                                          pview[:, 0:kn, :, 0:M])
                else:
                    nc.scalar.mul(sv[:, toff:toff + kn, :, 0:M],
                                  pview[:, 0:kn, :, 0:M], 1.0)
                if kg % 2 == 1 or kg == NG - 1:
                    ktot = kg * 2 + kn - ssc_k0
                    nc.sync.dma_start(
                        s4[:, ssc_k0:ssc_k0 + ktot],
                        sv[:, 0:ktot, :, 0:M])

            # ============ stage 4: out[b] = E @ S ============
            Sk0 = pool2.tile([M, M * M], BF16, tag="cstk")   # S_re [k,(x,z)]
            Sk1 = pool2.tile([M, M * M], BF16, tag="cstk")   # S_im
            nc.gpsimd.dma_start(
                Sk0[:].rearrange("k (x z) -> k x z", x=M), Sdram[:, 0])
            nc.gpsimd.dma_start(
                Sk1[:].rearrange("k (x z) -> k x z", x=M), Sdram[:, 1])
            ore = outre[b]   # [y, x, z] layout
            oim = outim[b]
            for xg in range(32):                              # groups of 4 x's
                xn = min(4, M - xg * 4)
                cw = xn * M
                c0 = Sk0[:, xg * 4 * M:xg * 4 * M + cw]
                c1 = Sk1[:, xg * 4 * M:xg * 4 * M + cw]
                if xg % 2 == 0:
                    sore = scr4.tile([M, 2 * CH], BF16, tag="scr4")
                    soim = scr4.tile([M, 2 * CH], BF16, tag="scr4")
                    so_x0 = xg * 4
                off = (xg % 2) * CH
                po = ps4.tile([M, 1024], F32, tag="ps")
                nc.tensor.matmul(po[:, 0:cw], exre[:], c0, start=True, stop=False)
                nc.tensor.matmul(po[:, 0:cw], eximn[:], c1, start=False, stop=True)
                nc.vector.tensor_copy(sore[:, off:off + cw], po[:, 0:cw])
                po = ps4.tile([M, 1024], F32, tag="ps")
                nc.tensor.matmul(po[:, 0:cw], exim[:], c0, start=True, stop=False)
                nc.tensor.matmul(po[:, 0:cw], exre[:], c1, start=False, stop=True)
                nc.scalar.mul(soim[:, off:off + cw], po[:, 0:cw], 1.0)
                if xg % 2 == 1:
                    xtot = xg * 4 + xn - so_x0
                    nc.sync.dma_start(
                        ore[:, so_x0:so_x0 + xtot],
                        sore[:, 0:off + cw].rearrange(
                            "y (x z) -> y x z", x=xtot))
                    nc.sync.dma_start(
                        oim[:, so_x0:so_x0 + xtot],
                        soim[:, 0:off + cw].rearrange(
                            "y (x z) -> y x z", x=xtot))

    nc.compile()
    return nc


def _get_program():
    global _PROG
    if _PROG is None:
        _PROG = _build_program()
    return _PROG


def _make_inmaps(f_re, f_im, g_re, g_im, wig):
    u, v = _factor_wig(np.asarray(wig, dtype=np.float32))
    # dstkM[m,l,k] = u[l,k,m]; dstkT[n,l,k] = v[l,k,n]
    dstkM = np.ascontiguousarray(u.transpose(2, 0, 1)).astype(NPBF)
    dstkT = np.ascontiguousarray(v.transpose(2, 0, 1)).astype(NPBF)
    ang = 2.0 * np.pi * np.outer(np.arange(M), np.arange(M)) / M
    exre = (np.cos(ang) / M).astype(NPBF)
    exim = (np.sin(ang) / M).astype(NPBF)
    eximn = (-np.sin(ang) / M).astype(NPBF)

    def fgt(x, sl, neg=False):
        t = np.asarray(x, dtype=np.float32)[sl]            # (BC, L, M)
        t = t.transpose(2, 0, 1)                           # (M, BC, L)
        if neg:
            t = -t
        return np.ascontiguousarray(t).astype(NPBF)

    in_maps = []
    for c in range(NCORES):
        sl = slice(c * BC, (c + 1) * BC)
        in_maps.append({
            "dstkM": dstkM, "dstkT": dstkT,
            "exre": exre, "exim": exim, "eximn": eximn,
            "fre": fgt(f_re, sl), "fim": fgt(f_im, sl),
            "gre": fgt(g_re, sl), "gimn": fgt(g_im, sl, neg=True),
        })
    return in_maps


def _assemble_output(results):
    out = np.empty((B, M, M, M), dtype=np.complex64)
    for c in range(NCORES):
        r = results[c]
        # device wrote [b, y, x, z]; reference order is [b, x, y, z]
        out[c * BC:(c + 1) * BC] = (
            r["outre"].astype(np.float32).transpose(0, 2, 1, 3)
            + 1j * r["outim"].astype(np.float32).transpose(0, 2, 1, 3))
    return out


def kernel(f_re, f_im, g_re, g_im, wig):
    nc = _get_program()
    in_maps = _make_inmaps(f_re, f_im, g_re, g_im, wig)
    res = run_bass_kernel_spmd(nc, in_maps, list(range(NCORES)))
    return _assemble_output(res.results)

